# revision 15
# baseline (speedup 1.0000x reference)
"""CatNet spiking CNN on 8x TRN2 NeuronCores (data-parallel over batch N=64).

Integrated spike-count formulation: for an IF neuron (soft reset, thresh 1)
with cumulative input P_t and cumulative spike count S_t:
    s_t = (P_t - S_{t-1} >= 1 - (t+1)*bias),   S_t = S_{t-1} + s_t.
Convs are linear, so the cumulative drive of a layer fed by spikes is
conv(S^prev_t): each timestep computes F = conv(S^prev_t) fresh on the PE,
then one fused DVE compare + one DVE add per layer.

This version restructures every conv pass to use the (nearly) full 128x128
PE array per streamed column, halving the number of rhs streaming passes vs
the per-tap small-tile formulation:
  - L1: the 3-term bf16 split of (w, x) is K-stacked into one 63-row
    cross-product pattern; two images are block-diagonal in one pass.
  - conv2: state tile holds [S_A; S_B; S_A; S_B]; per-tap lhsT holds
    hi/lo weight splits in the matching row blocks -> one full-array pass
    per tap covers both images at hi/lo precision.
  - conv3: per-image duplicated state [S; S] with lhsT [w_hi; w_lo]
    (K=128); the two images of a pair run column-concurrently.
Duplicated / interleaved state tiles are refreshed each timestep by
ScalarE/GpSimd interior copies from the canonical DVE-updated states.
fc1 weights are prefetched into SBUF during the conv loop and the Q
trajectories are streamed to DRAM per-timestep, eliminating the DMA tail.
"""
import numpy as np
import ml_dtypes

import concourse.bass as bass
import concourse.mybir as mybir
from concourse import tile
from concourse.bass_utils import run_bass_kernel_spmd
from concourse.tile_rust import add_dep_helper

F32 = mybir.dt.float32
BF16 = mybir.dt.bfloat16
BF = ml_dtypes.bfloat16
ALU = mybir.AluOpType
ACTF = mybir.ActivationFunctionType

NCORE = 8
NI = 8
T = 16
WP = 32                 # padded row stride (30 cols used)
HPAD = 30
PADPIX = HPAD * WP      # 960
NPIX = 784
NH = 392
TAPS = [(ky, kx) for ky in range(3) for kx in range(3)]
# (w-term, x-term) cross products covering the 3-term bf16 split
L1_PAIRS = [(0, 0), (1, 1), (2, 2), (1, 0), (0, 1), (2, 0), (0, 2)]
NL1 = 64  # 63-row pattern padded to 64 with a zero row (32-aligned DMA)


def _split_excess_waits(nc, maxw=1):
    """This walrus accepts only one sync-wait per instruction; hoist extras
    onto preceding same-engine nops."""
    ctr = [0]
    for bb in nc.m.functions[0].blocks:
        nl = []
        for inst in bb.instructions:
            w = list(inst.sync_info.on_wait) if inst.sync_info else []
            if len(w) > maxw:
                keep, exc = w[:maxw], w[maxw:]
                for cs in range(0, len(exc), maxw):
                    nop = mybir.InstNoOp(name=f"I-ws-{ctr[0]}", ins=[], outs=[])
                    ctr[0] += 1
                    nop.engine = inst.engine
                    nop.sync_info = mybir.SyncInfo(
                        on_wait=list(exc[cs:cs + maxw]), on_update=[])
                    nc.register_instruction(nop)
                    nl.append(nop)
                inst.sync_info = mybir.SyncInfo(
                    on_wait=list(keep), on_update=list(inst.sync_info.on_update))
            nl.append(inst)
        bb.instructions[:] = nl


def _rap(handle, offset, dims):
    """Raw access pattern on a DRAM tensor handle."""
    return bass.AP(handle, offset, [list(d) for d in dims])


def _pad3(ap_pad):
    """[P, 960] padded tile -> [P, 30, 32] view."""
    return ap_pad.rearrange("p (h w) -> p h w", w=WP)


def _interior(ap_pad):
    """[P, 960] padded tile -> [P, 28, 28] interior view."""
    return _pad3(ap_pad)[:, 1:29, 1:29]


def _cwin(ap_pad, ky, kx, h0):
    """Conv-tap window: out rows h0..h0+13 -> padded rows h0+ky.., cols kx.."""
    return _pad3(ap_pad)[:, h0 + ky:h0 + ky + 14, kx:kx + 28]


def _pwin(ap_pad, dy, dx):
    """Pool window: [P, 14, 14], elem (r,c) = padded (2r+1+dy, 2c+1+dx)."""
    return _pad3(ap_pad)[:, 1 + dy:1 + dy + 28:2, 1 + dx:1 + dx + 28:2]


def build_nc(nt=T, ni=NI):
    nc = bass.Bass()
    pn = ni // 2            # image pairs
    gn = ni // 4            # groups of 4 images

    xst = nc.dram_tensor("xst", [ni * nt, NL1, 2, NH], BF16, kind="ExternalInput")
    w1xq = nc.dram_tensor("w1xq", [128, 128], BF16, kind="ExternalInput")
    w1hq = nc.dram_tensor("w1hq", [128, 64], BF16, kind="ExternalInput")
    w2q = nc.dram_tensor("w2q", [128, 9, 128], BF16, kind="ExternalInput")
    w3q = nc.dram_tensor("w3q", [128, 9, 64], BF16, kind="ExternalInput")
    idq = nc.dram_tensor("idq", [128, 128], BF16, kind="ExternalInput")
    wf1s = nc.dram_tensor("wf1s", [98, 2, 128, 128], BF16, kind="ExternalInput")
    wf2s = nc.dram_tensor("wf2s", [128, 2, 10], BF16, kind="ExternalInput")
    thrX1 = nc.dram_tensor("thrX1", [128, nt], F32, kind="ExternalInput")
    thrH0 = nc.dram_tensor("thrH0", [128, nt], F32, kind="ExternalInput")
    thrH1 = nc.dram_tensor("thrH1", [128, nt], F32, kind="ExternalInput")
    thrH2 = nc.dram_tensor("thrH2", [128, nt], F32, kind="ExternalInput")
    thrF1 = nc.dram_tensor("thrF1", [128, nt], F32, kind="ExternalInput")
    bf2t = nc.dram_tensor("bf2t", [10, 1], F32, kind="ExternalInput")
    out = nc.dram_tensor("out", [ni, 10], F32, kind="ExternalOutput")

    with tile.TileContext(nc) as tc:
        with (
            tc.tile_pool(name="wpool", bufs=1) as wp,
            tc.tile_pool(name="state", bufs=1) as stp,
            tc.tile_pool(name="scratch", bufs=1) as scp,
            tc.tile_pool(name="psum", bufs=1, space="PSUM") as psp,
        ):
            # ---- weights / thresholds ----
            w1xs = wp.tile([128, 128], BF16, name="w1xs")
            nc.sync.dma_start(w1xs[:, :], w1xq[:, :])
            w1hs = wp.tile([128, 64], BF16, name="w1hs")
            nc.sync.dma_start(w1hs[:, :], w1hq[:, :])
            w2s = wp.tile([128, 9, 128], BF16, name="w2s")
            nc.sync.dma_start(w2s[:, :, :], w2q[:, :, :])
            w3s = wp.tile([128, 9, 64], BF16, name="w3s")
            nc.sync.dma_start(w3s[:, :, :], w3q[:, :, :])
            ids = wp.tile([128, 128], BF16, name="ids")
            nc.sync.dma_start(ids[:, :], idq[:, :])
            wf2l = wp.tile([128, 2, 10], BF16, name="wf2l")
            nc.sync.dma_start(wf2l[:, :, :], wf2s[:, :, :])
            tX1 = wp.tile([128, nt], F32, name="tX1")
            nc.sync.dma_start(tX1[:, :], thrX1[:, :])
            tH0 = wp.tile([128, nt], F32, name="tH0")
            nc.sync.dma_start(tH0[:, :], thrH0[:, :])
            tH1 = wp.tile([128, nt], F32, name="tH1")
            nc.sync.dma_start(tH1[:, :], thrH1[:, :])
            tH2 = wp.tile([128, nt], F32, name="tH2")
            nc.sync.dma_start(tH2[:, :], thrH2[:, :])
            tF1 = wp.tile([128, nt], F32, name="tF1")
            nc.sync.dma_start(tF1[:, :], thrF1[:, :])
            bf2l = wp.tile([10, 1], F32, name="bf2l")
            nc.sync.dma_start(bf2l[:, :], bf2t[:, :])
            # fc1 weights, prefetched in chunks inside the t-loop
            wf1t = wp.tile([128, 98, 2, 128], BF16, name="wf1t")

            # ---- persistent states (memsets split across engines) ----
            ms_engines = [nc.gpsimd, nc.vector]
            ms_ctr = [0]

            def ms(ap, v):
                ms_engines[ms_ctr[0] % 2].memset(ap, v)
                ms_ctr[0] += 1

            sX1P, d2t, sH1, d3a, d3b, sH2, qH3 = [], [], [], [], [], [], []
            for p in range(pn):
                t0_ = stp.tile([128, PADPIX], BF16, name=f"sX1P_{p}")
                ms(t0_[:, :], 0.0)
                sX1P.append(t0_)
                t1_ = stp.tile([128, PADPIX], BF16, name=f"d2_{p}")
                ms(t1_[:, :], 0.0)
                d2t.append(t1_)
                t2_ = stp.tile([128, NPIX], BF16, name=f"sH1_{p}")
                ms(t2_[:, :], 0.0)
                sH1.append(t2_)
                t3_ = stp.tile([128, PADPIX], BF16, name=f"d3a_{p}")
                ms(t3_[:, :], 0.0)
                d3a.append(t3_)
                t4_ = stp.tile([128, PADPIX], BF16, name=f"d3b_{p}")
                ms(t4_[:, :], 0.0)
                d3b.append(t4_)
                t5_ = stp.tile([128, PADPIX], BF16, name=f"sH2_{p}")
                ms(t5_[:, :], 0.0)
                sH2.append(t5_)
                t6_ = stp.tile([128, 196, nt + 1], BF16, name=f"qH3_{p}")
                ms(t6_[:, :, :], 1.0)   # Q = S + 1
                qH3.append(t6_)
            sH0S = []
            for g in range(gn):
                t7_ = stp.tile([128, NPIX], BF16, name=f"sH0S_{g}")
                ms(t7_[:, :], 0.0)
                sH0S.append(t7_)
            # fc1 rhs: Q trajectories transposed on-chip to [(j,c), px, i, t]
            rtall = stp.tile([128, 98, ni, nt], BF16, name="rtall")

            last_in_bank = {}   # chain-key -> last matmul of prior group

            def mm(key, fw, lhsT, rhs, st, sp, tp=(0, 0)):
                m = nc.tensor.matmul(fw, lhsT, rhs, start=st, stop=sp,
                                     tile_position=tp)
                if st and key in last_in_bank:
                    add_dep_helper(m.ins, last_in_bank[key].ins,
                                   sync=False, reason="psum group order")
                if sp:
                    last_in_bank[key] = m
                return m

            # ================= time loop =================
            fX1 = {}
            fH0 = {}
            fH1 = {}
            fH2 = {}
            for t in range(nt):
                # ---- input DMAs ----
                imst = {}
                for p in range(pn):
                    im = scp.tile([128, 2, NH], BF16, tag="imst",
                                  name=f"imst_{t}_{p}", bufs=6)
                    for e in range(2):
                        i = 2 * p + e
                        off = (i * nt + t) * NL1 * 2 * NH
                        nc.sync.dma_start(
                            im[64 * e:64 * e + 64, :, :],
                            _rap(xst, off, [[2 * NH, NL1], [NH, 2], [1, NH]]))
                    imst[p] = im
                # fc1 weight prefetch chunk
                px0, px1 = 7 * t, min(98, 7 * t + 7)
                for px in range(px0, px1):
                    nc.sync.dma_start(
                        wf1t[:, px, :, :],
                        wf1s[px, :, :, :].rearrange("a p m -> p a m"))

                # ---- L1 matmuls ----
                for p in range(pn):
                    for h in range(2):
                        fX1[(p, h)] = psp.tile([128, 512], F32, tag="F",
                                               name=f"fX1_{t}_{p}_{h}", bufs=6)
                        mm(("x1", p, h), fX1[(p, h)][:, 0:NH],
                           w1xs[:, :], imst[p][:, h, :], True, True)
                    g, e = p // 2, p % 2
                    if e == 0:
                        for h in range(2):
                            fH0[(g, h)] = psp.tile([128, 512], F32, tag="F",
                                                   name=f"fH0_{t}_{g}_{h}",
                                                   bufs=6)
                    for h in range(2):
                        mm(("h0", g, h, e),
                           fH0[(g, h)][64 * e:64 * e + 64, 0:NH],
                           w1hs[:, :], imst[p][:, h, :], True, True,
                           tp=(0, 64 * e))

                # ---- L1 post ----
                for p in range(pn):
                    sP = scp.tile([128, NPIX], BF16, tag="s",
                                  name=f"sx1_{t}_{p}", bufs=6)
                    for h in range(2):
                        nc.vector.scalar_tensor_tensor(
                            sP[:, NH * h:NH * h + NH].rearrange(
                                "p (a b) -> p a b", b=28),
                            fX1[(p, h)][:, 0:NH].rearrange(
                                "p (a b) -> p a b", b=28),
                            tX1[:, t:t + 1],
                            _pad3(sX1P[p][:, :])[:, 1 + 14 * h:15 + 14 * h, 1:29],
                            op0=ALU.subtract, op1=ALU.is_ge)
                    nc.vector.tensor_tensor(
                        _interior(sX1P[p][:, :]), _interior(sX1P[p][:, :]),
                        sP[:, :].rearrange("p (h w) -> p h w", w=28), ALU.add)
                for g in range(gn):
                    sP = scp.tile([128, NPIX], BF16, tag="s",
                                  name=f"sh0_{t}_{g}", bufs=6)
                    for h in range(2):
                        nc.vector.scalar_tensor_tensor(
                            sP[:, NH * h:NH * h + NH],
                            fH0[(g, h)][:, 0:NH],
                            tH0[:, t:t + 1],
                            sH0S[g][:, NH * h:NH * h + NH],
                            op0=ALU.subtract, op1=ALU.is_ge)
                    nc.gpsimd.tensor_tensor(
                        sH0S[g][:, :], sH0S[g][:, :], sP[:, :], ALU.add)
                # d2 refresh: [S_A; S_B; S_A; S_B] interior from canonical
                for p in range(pn):
                    g, e = p // 2, p % 2
                    src = sH0S[g][64 * e:64 * e + 64, :].rearrange(
                        "p (h w) -> p h w", w=28)
                    nc.scalar.copy(_interior(d2t[p][0:64, :]), src)
                    nc.scalar.copy(_interior(d2t[p][64:128, :]), src)

                # ---- h1: conv2(S_h0) + S_x1 ----
                for p in range(pn):
                    for h in range(2):
                        fH1[(p, h)] = psp.tile([128, 512], F32, tag="F",
                                               name=f"fH1_{t}_{p}_{h}", bufs=6)
                        fw = fH1[(p, h)][:, 0:NH]
                        key = ("h1", p, h)
                        mm(key, fw, ids[:, :],
                           _pad3(sX1P[p][:, :])[:, 1 + 14 * h:15 + 14 * h, 1:29],
                           True, False)
                        for k, (ky, kx) in enumerate(TAPS):
                            mm(key, fw, w2s[:, k, :],
                               _cwin(d2t[p][:, :], ky, kx, 14 * h),
                               False, k == 8)
                for p in range(pn):
                    sP = scp.tile([128, NPIX], BF16, tag="s",
                                  name=f"sh1_{t}_{p}", bufs=6)
                    for h in range(2):
                        nc.vector.scalar_tensor_tensor(
                            sP[:, NH * h:NH * h + NH],
                            fH1[(p, h)][:, 0:NH],
                            tH1[:, t:t + 1],
                            sH1[p][:, NH * h:NH * h + NH],
                            op0=ALU.subtract, op1=ALU.is_ge)
                    nc.gpsimd.tensor_tensor(
                        sH1[p][:, :], sH1[p][:, :], sP[:, :], ALU.add)
                # d3 refresh: per-image duplicated [S; S] interiors (ScalarE)
                for p in range(pn):
                    srcA = sH1[p][0:64, :].rearrange("p (h w) -> p h w", w=28)
                    srcB = sH1[p][64:128, :].rearrange("p (h w) -> p h w", w=28)
                    nc.scalar.copy(_interior(d3a[p][0:64, :]), srcA)
                    nc.scalar.copy(_interior(d3a[p][64:128, :]), srcA)
                    nc.scalar.copy(_interior(d3b[p][0:64, :]), srcB)
                    nc.scalar.copy(_interior(d3b[p][64:128, :]), srcB)

                # ---- h2: conv3(S_h1), hi/lo K-stacked, pair col-concurrent ----
                for p in range(pn):
                    for h in range(2):
                        fH2[(p, h)] = psp.tile([128, 512], F32, tag="F",
                                               name=f"fH2_{t}_{p}_{h}", bufs=6)
                        for k, (ky, kx) in enumerate(TAPS):
                            mm(("h2", p, h, 0),
                               fH2[(p, h)][0:64, 0:NH], w3s[:, k, :],
                               _cwin(d3a[p][:, :], ky, kx, 14 * h),
                               k == 0, k == 8, tp=(0, 0))
                            mm(("h2", p, h, 1),
                               fH2[(p, h)][64:128, 0:NH], w3s[:, k, :],
                               _cwin(d3b[p][:, :], ky, kx, 14 * h),
                               k == 0, k == 8, tp=(0, 64))
                for p in range(pn):
                    sP = scp.tile([128, NPIX], BF16, tag="s",
                                  name=f"sh2_{t}_{p}", bufs=6)
                    for h in range(2):
                        nc.vector.scalar_tensor_tensor(
                            sP[:, NH * h:NH * h + NH].rearrange(
                                "p (a b) -> p a b", b=28),
                            fH2[(p, h)][:, 0:NH].rearrange(
                                "p (a b) -> p a b", b=28),
                            tH2[:, t:t + 1],
                            _pad3(sH2[p][:, :])[:, 1 + 14 * h:15 + 14 * h, 1:29],
                            op0=ALU.subtract, op1=ALU.is_ge)
                    nc.vector.tensor_tensor(
                        _interior(sH2[p][:, :]), _interior(sH2[p][:, :]),
                        sP[:, :].rearrange("p (h w) -> p h w", w=28), ALU.add)

                # ---- h3: 2x2 sum pool (x1.1 folded into compare) ----
                for p in range(pn):
                    f3 = psp.tile([128, 512], F32, tag="F3",
                                  name=f"f3_{t}_{p}", bufs=2)
                    key = ("h3", p)
                    for wi, (dy, dx) in enumerate(
                            ((0, 0), (0, 1), (1, 0), (1, 1))):
                        mm(key, f3[:, 0:196], ids[:, :],
                           _pwin(sH2[p][:, :], dy, dx), wi == 0, wi == 3)
                    sP = scp.tile([128, NPIX], BF16, tag="s",
                                  name=f"sh3_{t}_{p}", bufs=6)
                    nc.vector.scalar_tensor_tensor(
                        sP[:, 0:196], f3[:, 0:196], 1.1,
                        qH3[p][:, :, t], op0=ALU.mult, op1=ALU.is_ge)
                    nc.vector.tensor_tensor(
                        qH3[p][:, :, t + 1], qH3[p][:, :, t],
                        sP[:, 0:196], ALU.add)
                    # transpose Q slice into the fc1 rhs buffer:
                    # rtall[64j+c, px, i, t] = Q[c, 2px+j] of image i=2p+e.
                    # (j==e: no partition shift -> DVE; j!=e: ScalarE)
                    for e in range(2):
                        for j in range(2):
                            src = qH3[p][64 * e:64 * e + 64, j:196:2, t + 1]
                            dst = rtall[64 * j:64 * j + 64, :, 2 * p + e, t]
                            if j == e:
                                nc.vector.tensor_copy(dst, src)
                            else:
                                nc.scalar.copy(dst, src)

            # ---------- fc1 ----------
            ff1 = psp.tile([128, 512], F32, tag="F3", name="ff1", bufs=2)
            for px in range(98):
                for hl in range(2):
                    nc.tensor.matmul(ff1[:, 0:ni * nt], wf1t[:, px, hl, :],
                                     rtall[:, px, :, :],
                                     start=(px == 0 and hl == 0),
                                     stop=(px == 97 and hl == 1))

            # f1 spike scan (Q_h3 includes +1 offset; folded into thrF1)
            sf1 = stp.tile([128, ni], BF16, name="sf1")
            nc.gpsimd.memset(sf1[:, :], 0.0)
            for t in range(nt):
                sPf = scp.tile([128, ni], BF16, tag="sf", name=f"sf_{t}", bufs=2)
                nc.vector.scalar_tensor_tensor(
                    sPf[:, :],
                    ff1[:, 0:ni * nt].rearrange("p (n t) -> p n t", t=nt)[:, :, t],
                    tF1[:, t:t + 1], sf1[:, :],
                    op0=ALU.subtract, op1=ALU.is_ge)
                nc.vector.tensor_tensor(sf1[:, :], sf1[:, :], sPf[:, :], ALU.add)

            # ---------- fc2 + readout ----------
            ff2 = psp.tile([128, 512], F32, tag="F3", name="ff2", bufs=2)
            for hl in range(2):
                nc.tensor.matmul(ff2[0:10, 0:ni], wf2l[:, hl, :], sf1[:, :],
                                 start=(hl == 0), stop=(hl == 1))
            osb = scp.tile([10, ni], F32, tag="osb", name="osb")
            nc.scalar.activation(osb[:, :], ff2[0:10, 0:ni], ACTF.Identity,
                                 bias=bf2l[:, :], scale=1.0 / nt)
            nc.sync.dma_start(out[:, :].rearrange("n o -> o n"), osb[:, :])

    _split_excess_waits(nc)
    return nc


# ---------------- host side ----------------

def _split(a):
    hi = np.asarray(a, np.float64).astype(BF)
    lo = (np.asarray(a, np.float64) - hi.astype(np.float64)).astype(BF)
    return hi, lo


def _split3(a):
    a = np.asarray(a, np.float64)
    p0 = a.astype(BF)
    r = a - p0.astype(np.float64)
    p1 = r.astype(BF)
    p2 = (r - p1.astype(np.float64)).astype(BF)
    return p0, p1, p2


def _prep_shared(w1a, b1a, w1, b1, w2, b2, w3, b3, wf1, bf1, wf2, bf2, nt=T):
    d = {}
    # L1: 63-row cross-product pattern of the 3-term split (padded to 64),
    # two images block-diagonal. w1xq: rows 0-62 -> cols 0-63 (img A),
    # rows 64-126 -> cols 64-127 (img B).
    a1 = _split3(w1a.reshape(64, 9).T)          # each [9, 64]
    w1xqa = np.zeros((128, 128), BF)
    for j, (wi, _) in enumerate(L1_PAIRS):
        w1xqa[9 * j:9 * j + 9, 0:64] = a1[wi]
        w1xqa[64 + 9 * j:64 + 9 * j + 9, 64:128] = a1[wi]
    d["w1xq"] = w1xqa
    h1v = _split3(w1.reshape(32, 9).T)          # each [9, 32]
    w1hqa = np.zeros((128, 64), BF)
    for j, (wi, _) in enumerate(L1_PAIRS):
        w1hqa[9 * j:9 * j + 9, 0:32] = h1v[wi]
        w1hqa[64 + 9 * j:64 + 9 * j + 9, 32:64] = h1v[wi]
    d["w1hq"] = w1hqa
    # conv2 [tap, 128, 128]: interleaved [S_A; S_B; S_A; S_B] row blocks
    a2 = np.transpose(w2.reshape(64, 32, 3, 3), (2, 3, 1, 0)).reshape(9, 32, 64)
    h2, l2 = _split(a2)
    w2qa = np.zeros((128, 9, 128), BF)
    for k in range(9):
        w2qa[0:32, k, 0:64] = h2[k]
        w2qa[64:96, k, 0:64] = l2[k]
        w2qa[32:64, k, 64:128] = h2[k]
        w2qa[96:128, k, 64:128] = l2[k]
    d["w2q"] = w2qa
    # conv3 [tap, 128, 64]: K-stacked [w_hi; w_lo] vs duplicated state [S; S]
    a3 = np.transpose(w3.reshape(64, 64, 3, 3), (2, 3, 1, 0)).reshape(9, 64, 64)
    h3v, l3v = _split(a3)
    w3qa = np.zeros((128, 9, 64), BF)
    w3qa[0:64, :, :] = np.transpose(h3v, (1, 0, 2))
    w3qa[64:128, :, :] = np.transpose(l3v, (1, 0, 2))
    d["w3q"] = w3qa
    d["idq"] = np.eye(128, dtype=BF)
    # fc1 tiles [pxh, hl, row=64j+c, m]
    wf1p = np.asarray(wf1, np.float64)                  # [128, 64, 14, 14]
    wf1f = wf1p.reshape(128, 64, 196)                   # px = 14*h + w
    wf1sa = np.zeros((98, 2, 128, 128), BF)
    hi1, lo1 = _split(wf1f)
    for pxh in range(98):
        for j in range(2):
            px = 2 * pxh + j
            wf1sa[pxh, 0, 64 * j:64 * j + 64, :] = hi1[:, :, px].T
            wf1sa[pxh, 1, 64 * j:64 * j + 64, :] = lo1[:, :, px].T
    d["wf1s"] = wf1sa
    h2v, l2v = _split(np.asarray(wf2, np.float64).T)    # [128, 10]
    wf2sa = np.zeros((128, 2, 10), BF)
    wf2sa[:, 0, :] = h2v
    wf2sa[:, 1, :] = l2v
    d["wf2s"] = wf2sa
    # thresholds [128, nt]
    tsteps = np.arange(1, nt + 1)
    b1a_t = 1.0 - tsteps[None, :] * np.asarray(b1a, np.float64)[:, None]
    thrX1a = np.ones((128, nt), np.float32)
    thrX1a[0:64] = b1a_t
    thrX1a[64:128] = b1a_t
    d["thrX1"] = thrX1a
    b1_t = 1.0 - tsteps[None, :] * np.asarray(b1, np.float64)[:, None]
    thrH0a = np.ones((128, nt), np.float32)
    for q in range(4):
        thrH0a[32 * q:32 * q + 32] = b1_t
    d["thrH0"] = thrH0a
    thrH1a = np.ones((128, nt), np.float32)
    b2d = np.asarray(b2, np.float64)
    thrH1a[0:64] = 1.0 - tsteps[None, :] * b2d[:, None]
    thrH1a[64:128] = 1.0 - tsteps[None, :] * b2d[:, None]
    d["thrH1"] = thrH1a
    thrH2a = np.ones((128, nt), np.float32)
    b3d = np.asarray(b3, np.float64)
    thrH2a[0:64] = 1.0 - tsteps[None, :] * b3d[:, None]
    thrH2a[64:128] = 1.0 - tsteps[None, :] * b3d[:, None]
    d["thrH2"] = thrH2a
    # f1: (F_comp = wf1p . Q) >= S + 1 + rowsum(wf1p) - (t+1)*bf1
    rs = wf1f.sum(axis=(1, 2))                          # [128]
    thrF1a = np.zeros((128, nt), np.float32)
    thrF1a[:, :] = (1.0 + rs[:, None]
                    - tsteps[None, :] * np.asarray(bf1, np.float64)[:, None])
    d["thrF1"] = thrF1a
    d["bf2t"] = np.asarray(bf2, np.float32).reshape(10, 1)
    return d


def _prep_x(xc, nt=T):
    """xc [ni, 1, 28, 28, nt] -> host im2col of the 3-term-split cumulative
    input with the 63-row cross-product x-pattern: [ni*nt, 63, 2, 392]."""
    ni = xc.shape[0]
    X = np.cumsum(np.asarray(xc, np.float64), axis=-1)[:, 0]   # [ni, 28, 28, nt]
    X = np.moveaxis(X, -1, 1)                                  # [ni, nt, 28, 28]
    pad = np.zeros((ni * nt, 30, 30), np.float64)
    pad[:, 1:29, 1:29] = X.reshape(ni * nt, 28, 28)
    col = np.zeros((ni * nt, 9, 2, NH), np.float64)
    for k, (ky, kx) in enumerate(TAPS):
        for h in range(2):
            col[:, k, h, :] = pad[:, 14 * h + ky:14 * h + ky + 14,
                                  kx:kx + 28].reshape(ni * nt, NH)
    xs = _split3(col)                                          # each [.., 9, 2, NH]
    outp = np.zeros((ni * nt, NL1, 2, NH), BF)                 # row 63 stays 0
    for j, (_, xi) in enumerate(L1_PAIRS):
        outp[:, 9 * j:9 * j + 9, :, :] = xs[xi]
    return outp


_NC_CACHE = {}


def kernel(x, w1a, b1a, w1, b1, w2, b2, w3, b3, wf1, bf1, wf2, bf2):
    x = np.asarray(x)
    n_total = x.shape[0]
    ni = n_total // NCORE
    key = (ni, T)
    if key not in _NC_CACHE:
        _NC_CACHE[key] = build_nc(T, ni)
    nc = _NC_CACHE[key]

    shared = _prep_shared(w1a, b1a, w1, b1, w2, b2, w3, b3, wf1, bf1, wf2, bf2)
    in_maps = []
    for c in range(NCORE):
        m = dict(shared)
        m["xst"] = _prep_x(x[c * ni:(c + 1) * ni])
        in_maps.append(m)

    res = run_bass_kernel_spmd(nc, in_maps, list(range(NCORE))).results
    return np.concatenate([res[c]["out"] for c in range(NCORE)], axis=0)


# revision 20
# speedup vs baseline: 1.0761x; 1.0761x over previous
"""CatNet spiking CNN on 8x TRN2 NeuronCores (data-parallel over batch N=64).

Integrated spike-count formulation: for an IF neuron (soft reset, thresh 1)
with cumulative input P_t and cumulative spike count S_t:
    s_t = (P_t - S_{t-1} >= 1 - (t+1)*bias),   S_t = S_{t-1} + s_t.
Convs are linear, so the cumulative drive of a layer fed by spikes is
conv(S^prev_t): each timestep computes F = conv(S^prev_t) fresh on the PE,
then one fused DVE compare + one DVE add per layer.

This version restructures every conv pass to use the (nearly) full 128x128
PE array per streamed column, halving the number of rhs streaming passes vs
the per-tap small-tile formulation:
  - L1: the 3-term bf16 split of (w, x) is K-stacked into one 63-row
    cross-product pattern; two images are block-diagonal in one pass.
  - conv2: state tile holds [S_A; S_B; S_A; S_B]; per-tap lhsT holds
    hi/lo weight splits in the matching row blocks -> one full-array pass
    per tap covers both images at hi/lo precision.
  - conv3: per-image duplicated state [S; S] with lhsT [w_hi; w_lo]
    (K=128); the two images of a pair run column-concurrently.
Duplicated / interleaved state tiles are refreshed each timestep by
ScalarE/GpSimd interior copies from the canonical DVE-updated states.
fc1 weights are prefetched into SBUF during the conv loop and the Q
trajectories are streamed to DRAM per-timestep, eliminating the DMA tail.
"""
import numpy as np
import ml_dtypes

import concourse.bass as bass
import concourse.mybir as mybir
from concourse import tile
from concourse.bass_utils import run_bass_kernel_spmd
from concourse.tile_rust import add_dep_helper

F32 = mybir.dt.float32
BF16 = mybir.dt.bfloat16
BF = ml_dtypes.bfloat16
ALU = mybir.AluOpType
ACTF = mybir.ActivationFunctionType

NCORE = 8
NI = 8
T = 16
WP = 32                 # padded row stride (30 cols used)
HPAD = 30
PADPIX = HPAD * WP      # 960
NPIX = 784
NH = 392
TAPS = [(ky, kx) for ky in range(3) for kx in range(3)]
# (w-term, x-term) cross products covering the 3-term bf16 split
L1_PAIRS = [(0, 0), (1, 1), (2, 2), (1, 0), (0, 1), (2, 0), (0, 2)]
NL1 = 64  # 63-row pattern padded to 64 with a zero row (32-aligned DMA)


def _split_excess_waits(nc, maxw=1):
    """This walrus accepts only one sync-wait per instruction; hoist extras
    onto preceding same-engine nops."""
    ctr = [0]
    for bb in nc.m.functions[0].blocks:
        nl = []
        for inst in bb.instructions:
            w = list(inst.sync_info.on_wait) if inst.sync_info else []
            if len(w) > maxw:
                keep, exc = w[:maxw], w[maxw:]
                for cs in range(0, len(exc), maxw):
                    nop = mybir.InstNoOp(name=f"I-ws-{ctr[0]}", ins=[], outs=[])
                    ctr[0] += 1
                    nop.engine = inst.engine
                    nop.sync_info = mybir.SyncInfo(
                        on_wait=list(exc[cs:cs + maxw]), on_update=[])
                    nc.register_instruction(nop)
                    nl.append(nop)
                inst.sync_info = mybir.SyncInfo(
                    on_wait=list(keep), on_update=list(inst.sync_info.on_update))
            nl.append(inst)
        bb.instructions[:] = nl


def _rap(handle, offset, dims):
    """Raw access pattern on a DRAM tensor handle."""
    return bass.AP(handle, offset, [list(d) for d in dims])


def _pad3(ap_pad):
    """[P, 960] padded tile -> [P, 30, 32] view."""
    return ap_pad.rearrange("p (h w) -> p h w", w=WP)


def _interior(ap_pad):
    """[P, 960] padded tile -> [P, 28, 28] interior view."""
    return _pad3(ap_pad)[:, 1:29, 1:29]


def _cwin(ap_pad, ky, kx, h0):
    """Conv-tap window: out rows h0..h0+13 -> padded rows h0+ky.., cols kx.."""
    return _pad3(ap_pad)[:, h0 + ky:h0 + ky + 14, kx:kx + 28]


def _pwin(ap_pad, dy, dx):
    """Pool window: [P, 14, 14], elem (r,c) = padded (2r+1+dy, 2c+1+dx)."""
    return _pad3(ap_pad)[:, 1 + dy:1 + dy + 28:2, 1 + dx:1 + dx + 28:2]


def build_nc(nt=T, ni=NI):
    nc = bass.Bass()
    pn = ni // 2            # image pairs
    gn = ni // 4            # groups of 4 images

    xst = nc.dram_tensor("xst", [ni * nt, NL1, 2, NH], BF16, kind="ExternalInput")
    w1xq = nc.dram_tensor("w1xq", [128, 128], BF16, kind="ExternalInput")
    w1hq = nc.dram_tensor("w1hq", [128, 64], BF16, kind="ExternalInput")
    w2q = nc.dram_tensor("w2q", [128, 9, 128], BF16, kind="ExternalInput")
    w3q = nc.dram_tensor("w3q", [128, 9, 64], BF16, kind="ExternalInput")
    idq = nc.dram_tensor("idq", [128, 128], BF16, kind="ExternalInput")
    wf1s = nc.dram_tensor("wf1s", [98, 2, 128, 128], BF16, kind="ExternalInput")
    wf2s = nc.dram_tensor("wf2s", [128, 2, 10], BF16, kind="ExternalInput")
    thrX1 = nc.dram_tensor("thrX1", [128, nt], F32, kind="ExternalInput")
    thrH0 = nc.dram_tensor("thrH0", [128, nt], F32, kind="ExternalInput")
    thrH1 = nc.dram_tensor("thrH1", [128, nt], F32, kind="ExternalInput")
    thrH2 = nc.dram_tensor("thrH2", [128, nt], F32, kind="ExternalInput")
    thrF1 = nc.dram_tensor("thrF1", [128, nt], F32, kind="ExternalInput")
    bf2t = nc.dram_tensor("bf2t", [10, 1], F32, kind="ExternalInput")
    out = nc.dram_tensor("out", [ni, 10], F32, kind="ExternalOutput")

    with tile.TileContext(nc) as tc:
        with (
            tc.tile_pool(name="wpool", bufs=1) as wp,
            tc.tile_pool(name="state", bufs=1) as stp,
            tc.tile_pool(name="scratch", bufs=1) as scp,
            tc.tile_pool(name="psum", bufs=1, space="PSUM") as psp,
        ):
            # ---- weights / thresholds ----
            w1xs = wp.tile([128, 128], BF16, name="w1xs")
            nc.sync.dma_start(w1xs[:, :], w1xq[:, :])
            w1hs = wp.tile([128, 64], BF16, name="w1hs")
            nc.sync.dma_start(w1hs[:, :], w1hq[:, :])
            w2s = wp.tile([128, 9, 128], BF16, name="w2s")
            nc.sync.dma_start(w2s[:, :, :], w2q[:, :, :])
            w3s = wp.tile([128, 9, 64], BF16, name="w3s")
            nc.sync.dma_start(w3s[:, :, :], w3q[:, :, :])
            ids = wp.tile([128, 128], BF16, name="ids")
            nc.sync.dma_start(ids[:, :], idq[:, :])
            wf2l = wp.tile([128, 2, 10], BF16, name="wf2l")
            nc.sync.dma_start(wf2l[:, :, :], wf2s[:, :, :])
            tX1 = wp.tile([128, nt], F32, name="tX1")
            nc.sync.dma_start(tX1[:, :], thrX1[:, :])
            tH0 = wp.tile([128, nt], F32, name="tH0")
            nc.sync.dma_start(tH0[:, :], thrH0[:, :])
            tH1 = wp.tile([128, nt], F32, name="tH1")
            nc.sync.dma_start(tH1[:, :], thrH1[:, :])
            tH2 = wp.tile([128, nt], F32, name="tH2")
            nc.sync.dma_start(tH2[:, :], thrH2[:, :])
            tF1 = wp.tile([128, nt], F32, name="tF1")
            nc.sync.dma_start(tF1[:, :], thrF1[:, :])
            bf2l = wp.tile([10, 1], F32, name="bf2l")
            nc.sync.dma_start(bf2l[:, :], bf2t[:, :])
            # fc1 weights, prefetched in chunks inside the t-loop
            wf1t = wp.tile([128, 98, 2, 128], BF16, name="wf1t")

            # ---- persistent states (memsets split across engines) ----
            ms_engines = [nc.gpsimd, nc.vector]
            ms_ctr = [0]

            def ms(ap, v):
                ms_engines[ms_ctr[0] % 2].memset(ap, v)
                ms_ctr[0] += 1

            sX1P, d2t, sH1, d3a, d3b, sH2, qH3 = [], [], [], [], [], [], []
            for p in range(pn):
                t0_ = stp.tile([128, PADPIX], BF16, name=f"sX1P_{p}")
                ms(t0_[:, :], 0.0)
                sX1P.append(t0_)
                t1_ = stp.tile([128, PADPIX], BF16, name=f"d2_{p}")
                ms(t1_[:, :], 0.0)
                d2t.append(t1_)
                t2_ = stp.tile([128, NPIX], BF16, name=f"sH1_{p}")
                ms(t2_[:, :], 0.0)
                sH1.append(t2_)
                t3_ = stp.tile([128, PADPIX], BF16, name=f"d3a_{p}")
                ms(t3_[:, :], 0.0)
                d3a.append(t3_)
                t4_ = stp.tile([128, PADPIX], BF16, name=f"d3b_{p}")
                ms(t4_[:, :], 0.0)
                d3b.append(t4_)
                t5_ = stp.tile([128, PADPIX], BF16, name=f"sH2_{p}")
                ms(t5_[:, :], 0.0)
                sH2.append(t5_)
                t6_ = stp.tile([128, 196, nt + 1], BF16, name=f"qH3_{p}")
                ms(t6_[:, :, :], 1.0)   # Q = S + 1
                qH3.append(t6_)
            sH0S = []
            for g in range(gn):
                t7_ = stp.tile([128, NPIX], BF16, name=f"sH0S_{g}")
                ms(t7_[:, :], 0.0)
                sH0S.append(t7_)
            # fc1 rhs: Q trajectories transposed on-chip to [(j,c), px, i, t]
            rtall = stp.tile([128, 98, ni, nt], BF16, name="rtall")

            last_in_bank = {}   # chain-key -> last matmul of prior group

            def mm(key, fw, lhsT, rhs, st, sp, tp=(0, 0)):
                m = nc.tensor.matmul(fw, lhsT, rhs, start=st, stop=sp,
                                     tile_position=tp)
                if st and key in last_in_bank:
                    add_dep_helper(m.ins, last_in_bank[key].ins,
                                   sync=False, reason="psum group order")
                if sp:
                    last_in_bank[key] = m
                return m

            # ================= time loop =================
            fX1 = {}
            fH0 = {}
            fH1 = {}
            fH2 = {}
            for t in range(nt):
                # ---- input DMAs ----
                imst = {}
                for p in range(pn):
                    im = scp.tile([128, 2, NH], BF16, tag="imst",
                                  name=f"imst_{t}_{p}", bufs=6)
                    for e in range(2):
                        i = 2 * p + e
                        off = (i * nt + t) * NL1 * 2 * NH
                        nc.sync.dma_start(
                            im[64 * e:64 * e + 64, :, :],
                            _rap(xst, off, [[2 * NH, NL1], [NH, 2], [1, NH]]))
                    imst[p] = im
                # fc1 weight prefetch chunk
                px0, px1 = 7 * t, min(98, 7 * t + 7)
                for px in range(px0, px1):
                    nc.sync.dma_start(
                        wf1t[:, px, :, :],
                        wf1s[px, :, :, :].rearrange("a p m -> p a m"))

                # ---- L1 matmuls ----
                for p in range(pn):
                    for h in range(2):
                        # stagger PSUM pressure: pairs 2,3 draw from the F3
                        # pool whose previous consumers (pool compares) are
                        # long done, avoiding an L1 stall on DVE backlog
                        tag = "F" if p < 2 else "F3"
                        fX1[(p, h)] = psp.tile([128, 512], F32, tag=tag,
                                               name=f"fX1_{t}_{p}_{h}",
                                               bufs=6 if p < 2 else 2)
                        mm(("x1", p, h), fX1[(p, h)][:, 0:NH],
                           w1xs[:, :], imst[p][:, h, :], True, True)
                    g, e = p // 2, p % 2
                    if e == 0:
                        for h in range(2):
                            fH0[(g, h)] = psp.tile([128, 512], F32, tag="F",
                                                   name=f"fH0_{t}_{g}_{h}",
                                                   bufs=6)
                    for h in range(2):
                        mm(("h0", g, h, e),
                           fH0[(g, h)][64 * e:64 * e + 64, 0:NH],
                           w1hs[:, :], imst[p][:, h, :], True, True,
                           tp=(0, 64 * e))

                # ---- L1 post ----
                for p in range(pn):
                    sP = scp.tile([128, NPIX], BF16, tag="s",
                                  name=f"sx1_{t}_{p}", bufs=6)
                    for h in range(2):
                        nc.vector.scalar_tensor_tensor(
                            sP[:, NH * h:NH * h + NH].rearrange(
                                "p (a b) -> p a b", b=28),
                            fX1[(p, h)][:, 0:NH].rearrange(
                                "p (a b) -> p a b", b=28),
                            tX1[:, t:t + 1],
                            _pad3(sX1P[p][:, :])[:, 1 + 14 * h:15 + 14 * h, 1:29],
                            op0=ALU.subtract, op1=ALU.is_ge)
                    nc.vector.tensor_tensor(
                        _interior(sX1P[p][:, :]), _interior(sX1P[p][:, :]),
                        sP[:, :].rearrange("p (h w) -> p h w", w=28), ALU.add)
                for g in range(gn):
                    sP = scp.tile([128, NPIX], BF16, tag="s",
                                  name=f"sh0_{t}_{g}", bufs=6)
                    for h in range(2):
                        nc.vector.scalar_tensor_tensor(
                            sP[:, NH * h:NH * h + NH],
                            fH0[(g, h)][:, 0:NH],
                            tH0[:, t:t + 1],
                            sH0S[g][:, NH * h:NH * h + NH],
                            op0=ALU.subtract, op1=ALU.is_ge)
                    nc.vector.tensor_tensor(
                        sH0S[g][:, :], sH0S[g][:, :], sP[:, :], ALU.add)
                # d2 refresh: [S_A; S_B; S_A; S_B] interior from canonical
                for p in range(pn):
                    g, e = p // 2, p % 2
                    src = sH0S[g][64 * e:64 * e + 64, :].rearrange(
                        "p (h w) -> p h w", w=28)
                    nc.scalar.copy(_interior(d2t[p][0:64, :]), src)
                    nc.scalar.copy(_interior(d2t[p][64:128, :]), src)

                # ---- h1: conv2(S_h0) + S_x1 ----
                for p in range(pn):
                    for h in range(2):
                        fH1[(p, h)] = psp.tile([128, 512], F32, tag="F",
                                               name=f"fH1_{t}_{p}_{h}", bufs=6)
                        fw = fH1[(p, h)][:, 0:NH]
                        key = ("h1", p, h)
                        mm(key, fw, ids[:, :],
                           _pad3(sX1P[p][:, :])[:, 1 + 14 * h:15 + 14 * h, 1:29],
                           True, False)
                        for k, (ky, kx) in enumerate(TAPS):
                            mm(key, fw, w2s[:, k, :],
                               _cwin(d2t[p][:, :], ky, kx, 14 * h),
                               False, k == 8)
                for p in range(pn):
                    sP = scp.tile([128, NPIX], BF16, tag="s",
                                  name=f"sh1_{t}_{p}", bufs=6)
                    for h in range(2):
                        nc.vector.scalar_tensor_tensor(
                            sP[:, NH * h:NH * h + NH],
                            fH1[(p, h)][:, 0:NH],
                            tH1[:, t:t + 1],
                            sH1[p][:, NH * h:NH * h + NH],
                            op0=ALU.subtract, op1=ALU.is_ge)
                    nc.vector.tensor_tensor(
                        sH1[p][:, :], sH1[p][:, :], sP[:, :], ALU.add)
                # d3 refresh: per-image duplicated [S; S] interiors (ScalarE)
                for p in range(pn):
                    srcA = sH1[p][0:64, :].rearrange("p (h w) -> p h w", w=28)
                    srcB = sH1[p][64:128, :].rearrange("p (h w) -> p h w", w=28)
                    nc.scalar.copy(_interior(d3a[p][0:64, :]), srcA)
                    nc.scalar.copy(_interior(d3a[p][64:128, :]), srcA)
                    nc.scalar.copy(_interior(d3b[p][0:64, :]), srcB)
                    nc.scalar.copy(_interior(d3b[p][64:128, :]), srcB)

                # ---- h2: conv3(S_h1), hi/lo K-stacked, pair col-concurrent ----
                for p in range(pn):
                    for h in range(2):
                        fH2[(p, h)] = psp.tile([128, 512], F32, tag="F",
                                               name=f"fH2_{t}_{p}_{h}", bufs=6)
                        for k, (ky, kx) in enumerate(TAPS):
                            mm(("h2", p, h, 0),
                               fH2[(p, h)][0:64, 0:NH], w3s[:, k, :],
                               _cwin(d3a[p][:, :], ky, kx, 14 * h),
                               k == 0, k == 8, tp=(0, 0))
                            mm(("h2", p, h, 1),
                               fH2[(p, h)][64:128, 0:NH], w3s[:, k, :],
                               _cwin(d3b[p][:, :], ky, kx, 14 * h),
                               k == 0, k == 8, tp=(0, 64))
                for p in range(pn):
                    sP = scp.tile([128, NPIX], BF16, tag="s",
                                  name=f"sh2_{t}_{p}", bufs=6)
                    for h in range(2):
                        nc.vector.scalar_tensor_tensor(
                            sP[:, NH * h:NH * h + NH].rearrange(
                                "p (a b) -> p a b", b=28),
                            fH2[(p, h)][:, 0:NH].rearrange(
                                "p (a b) -> p a b", b=28),
                            tH2[:, t:t + 1],
                            _pad3(sH2[p][:, :])[:, 1 + 14 * h:15 + 14 * h, 1:29],
                            op0=ALU.subtract, op1=ALU.is_ge)
                    nc.vector.tensor_tensor(
                        _interior(sH2[p][:, :]), _interior(sH2[p][:, :]),
                        sP[:, :].rearrange("p (h w) -> p h w", w=28), ALU.add)

                # ---- h3: 2x2 sum pool (x1.1 folded into compare) ----
                for p in range(pn):
                    f3 = psp.tile([128, 512], F32, tag="F3",
                                  name=f"f3_{t}_{p}", bufs=2)
                    key = ("h3", p)
                    for wi, (dy, dx) in enumerate(
                            ((0, 0), (0, 1), (1, 0), (1, 1))):
                        mm(key, f3[:, 0:196], ids[:, :],
                           _pwin(sH2[p][:, :], dy, dx), wi == 0, wi == 3)
                    sP = scp.tile([128, NPIX], BF16, tag="s",
                                  name=f"sh3_{t}_{p}", bufs=6)
                    nc.vector.scalar_tensor_tensor(
                        sP[:, 0:196], f3[:, 0:196], 1.1,
                        qH3[p][:, :, t], op0=ALU.mult, op1=ALU.is_ge)
                    nc.vector.tensor_tensor(
                        qH3[p][:, :, t + 1], qH3[p][:, :, t],
                        sP[:, 0:196], ALU.add)
                    # transpose Q slice into the fc1 rhs buffer:
                    # rtall[64j+c, px, i, t] = Q[c, 2px+j] of image i=2p+e.
                    # Latency-insensitive (only fc1 reads it) -> GpSimd.
                    for e in range(2):
                        for j in range(2):
                            src = qH3[p][64 * e:64 * e + 64, j:196:2, t + 1]
                            dst = rtall[64 * j:64 * j + 64, :, 2 * p + e, t]
                            nc.gpsimd.tensor_copy(dst, src)

            # ---------- fc1 ----------
            ff1 = psp.tile([128, 512], F32, tag="F3", name="ff1", bufs=2)
            for px in range(98):
                for hl in range(2):
                    nc.tensor.matmul(ff1[:, 0:ni * nt], wf1t[:, px, hl, :],
                                     rtall[:, px, :, :],
                                     start=(px == 0 and hl == 0),
                                     stop=(px == 97 and hl == 1))

            # f1 spike scan (Q_h3 includes +1 offset; folded into thrF1)
            sf1 = stp.tile([128, ni], BF16, name="sf1")
            nc.gpsimd.memset(sf1[:, :], 0.0)
            for t in range(nt):
                sPf = scp.tile([128, ni], BF16, tag="sf", name=f"sf_{t}", bufs=2)
                nc.vector.scalar_tensor_tensor(
                    sPf[:, :],
                    ff1[:, 0:ni * nt].rearrange("p (n t) -> p n t", t=nt)[:, :, t],
                    tF1[:, t:t + 1], sf1[:, :],
                    op0=ALU.subtract, op1=ALU.is_ge)
                nc.vector.tensor_tensor(sf1[:, :], sf1[:, :], sPf[:, :], ALU.add)

            # ---------- fc2 + readout ----------
            ff2 = psp.tile([128, 512], F32, tag="F3", name="ff2", bufs=2)
            for hl in range(2):
                nc.tensor.matmul(ff2[0:10, 0:ni], wf2l[:, hl, :], sf1[:, :],
                                 start=(hl == 0), stop=(hl == 1))
            osb = scp.tile([10, ni], F32, tag="osb", name="osb")
            nc.scalar.activation(osb[:, :], ff2[0:10, 0:ni], ACTF.Identity,
                                 bias=bf2l[:, :], scale=1.0 / nt)
            nc.sync.dma_start(out[:, :].rearrange("n o -> o n"), osb[:, :])

    _split_excess_waits(nc)
    return nc


# ---------------- host side ----------------

def _split(a):
    hi = np.asarray(a, np.float64).astype(BF)
    lo = (np.asarray(a, np.float64) - hi.astype(np.float64)).astype(BF)
    return hi, lo


def _split3(a):
    a = np.asarray(a, np.float64)
    p0 = a.astype(BF)
    r = a - p0.astype(np.float64)
    p1 = r.astype(BF)
    p2 = (r - p1.astype(np.float64)).astype(BF)
    return p0, p1, p2


def _prep_shared(w1a, b1a, w1, b1, w2, b2, w3, b3, wf1, bf1, wf2, bf2, nt=T):
    d = {}
    # L1: 63-row cross-product pattern of the 3-term split (padded to 64),
    # two images block-diagonal. w1xq: rows 0-62 -> cols 0-63 (img A),
    # rows 64-126 -> cols 64-127 (img B).
    a1 = _split3(w1a.reshape(64, 9).T)          # each [9, 64]
    w1xqa = np.zeros((128, 128), BF)
    for j, (wi, _) in enumerate(L1_PAIRS):
        w1xqa[9 * j:9 * j + 9, 0:64] = a1[wi]
        w1xqa[64 + 9 * j:64 + 9 * j + 9, 64:128] = a1[wi]
    d["w1xq"] = w1xqa
    h1v = _split3(w1.reshape(32, 9).T)          # each [9, 32]
    w1hqa = np.zeros((128, 64), BF)
    for j, (wi, _) in enumerate(L1_PAIRS):
        w1hqa[9 * j:9 * j + 9, 0:32] = h1v[wi]
        w1hqa[64 + 9 * j:64 + 9 * j + 9, 32:64] = h1v[wi]
    d["w1hq"] = w1hqa
    # conv2 [tap, 128, 128]: interleaved [S_A; S_B; S_A; S_B] row blocks
    a2 = np.transpose(w2.reshape(64, 32, 3, 3), (2, 3, 1, 0)).reshape(9, 32, 64)
    h2, l2 = _split(a2)
    w2qa = np.zeros((128, 9, 128), BF)
    for k in range(9):
        w2qa[0:32, k, 0:64] = h2[k]
        w2qa[64:96, k, 0:64] = l2[k]
        w2qa[32:64, k, 64:128] = h2[k]
        w2qa[96:128, k, 64:128] = l2[k]
    d["w2q"] = w2qa
    # conv3 [tap, 128, 64]: K-stacked [w_hi; w_lo] vs duplicated state [S; S]
    a3 = np.transpose(w3.reshape(64, 64, 3, 3), (2, 3, 1, 0)).reshape(9, 64, 64)
    h3v, l3v = _split(a3)
    w3qa = np.zeros((128, 9, 64), BF)
    w3qa[0:64, :, :] = np.transpose(h3v, (1, 0, 2))
    w3qa[64:128, :, :] = np.transpose(l3v, (1, 0, 2))
    d["w3q"] = w3qa
    d["idq"] = np.eye(128, dtype=BF)
    # fc1 tiles [pxh, hl, row=64j+c, m]
    wf1p = np.asarray(wf1, np.float64)                  # [128, 64, 14, 14]
    wf1f = wf1p.reshape(128, 64, 196)                   # px = 14*h + w
    wf1sa = np.zeros((98, 2, 128, 128), BF)
    hi1, lo1 = _split(wf1f)
    for pxh in range(98):
        for j in range(2):
            px = 2 * pxh + j
            wf1sa[pxh, 0, 64 * j:64 * j + 64, :] = hi1[:, :, px].T
            wf1sa[pxh, 1, 64 * j:64 * j + 64, :] = lo1[:, :, px].T
    d["wf1s"] = wf1sa
    h2v, l2v = _split(np.asarray(wf2, np.float64).T)    # [128, 10]
    wf2sa = np.zeros((128, 2, 10), BF)
    wf2sa[:, 0, :] = h2v
    wf2sa[:, 1, :] = l2v
    d["wf2s"] = wf2sa
    # thresholds [128, nt]
    tsteps = np.arange(1, nt + 1)
    b1a_t = 1.0 - tsteps[None, :] * np.asarray(b1a, np.float64)[:, None]
    thrX1a = np.ones((128, nt), np.float32)
    thrX1a[0:64] = b1a_t
    thrX1a[64:128] = b1a_t
    d["thrX1"] = thrX1a
    b1_t = 1.0 - tsteps[None, :] * np.asarray(b1, np.float64)[:, None]
    thrH0a = np.ones((128, nt), np.float32)
    for q in range(4):
        thrH0a[32 * q:32 * q + 32] = b1_t
    d["thrH0"] = thrH0a
    thrH1a = np.ones((128, nt), np.float32)
    b2d = np.asarray(b2, np.float64)
    thrH1a[0:64] = 1.0 - tsteps[None, :] * b2d[:, None]
    thrH1a[64:128] = 1.0 - tsteps[None, :] * b2d[:, None]
    d["thrH1"] = thrH1a
    thrH2a = np.ones((128, nt), np.float32)
    b3d = np.asarray(b3, np.float64)
    thrH2a[0:64] = 1.0 - tsteps[None, :] * b3d[:, None]
    thrH2a[64:128] = 1.0 - tsteps[None, :] * b3d[:, None]
    d["thrH2"] = thrH2a
    # f1: (F_comp = wf1p . Q) >= S + 1 + rowsum(wf1p) - (t+1)*bf1
    rs = wf1f.sum(axis=(1, 2))                          # [128]
    thrF1a = np.zeros((128, nt), np.float32)
    thrF1a[:, :] = (1.0 + rs[:, None]
                    - tsteps[None, :] * np.asarray(bf1, np.float64)[:, None])
    d["thrF1"] = thrF1a
    d["bf2t"] = np.asarray(bf2, np.float32).reshape(10, 1)
    return d


def _prep_x(xc, nt=T):
    """xc [ni, 1, 28, 28, nt] -> host im2col of the 3-term-split cumulative
    input with the 63-row cross-product x-pattern: [ni*nt, 63, 2, 392]."""
    ni = xc.shape[0]
    X = np.cumsum(np.asarray(xc, np.float64), axis=-1)[:, 0]   # [ni, 28, 28, nt]
    X = np.moveaxis(X, -1, 1)                                  # [ni, nt, 28, 28]
    pad = np.zeros((ni * nt, 30, 30), np.float64)
    pad[:, 1:29, 1:29] = X.reshape(ni * nt, 28, 28)
    col = np.zeros((ni * nt, 9, 2, NH), np.float64)
    for k, (ky, kx) in enumerate(TAPS):
        for h in range(2):
            col[:, k, h, :] = pad[:, 14 * h + ky:14 * h + ky + 14,
                                  kx:kx + 28].reshape(ni * nt, NH)
    xs = _split3(col)                                          # each [.., 9, 2, NH]
    outp = np.zeros((ni * nt, NL1, 2, NH), BF)                 # row 63 stays 0
    for j, (_, xi) in enumerate(L1_PAIRS):
        outp[:, 9 * j:9 * j + 9, :, :] = xs[xi]
    return outp


_NC_CACHE = {}


def kernel(x, w1a, b1a, w1, b1, w2, b2, w3, b3, wf1, bf1, wf2, bf2):
    x = np.asarray(x)
    n_total = x.shape[0]
    ni = n_total // NCORE
    key = (ni, T)
    if key not in _NC_CACHE:
        _NC_CACHE[key] = build_nc(T, ni)
    nc = _NC_CACHE[key]

    shared = _prep_shared(w1a, b1a, w1, b1, w2, b2, w3, b3, wf1, bf1, wf2, bf2)
    in_maps = []
    for c in range(NCORE):
        m = dict(shared)
        m["xst"] = _prep_x(x[c * ni:(c + 1) * ni])
        in_maps.append(m)

    res = run_bass_kernel_spmd(nc, in_maps, list(range(NCORE))).results
    return np.concatenate([res[c]["out"] for c in range(NCORE)], axis=0)
